# revision 11
# baseline (speedup 1.0000x reference)
"""Trainium2 Bass kernel for nn_AttLayer (attention pooling).

Reference computation (per sample b):
    uit = tanh(x @ W + b)            # [T, D]
    ait = uit @ u                    # [T]
    a   = exp(ait); a /= (sum(a) + 1e-7)
    out = a @ x                      # [D]

Sharding: data-parallel over batch B=32 across 8 cores (4 samples/core);
W/b/u replicated. No cross-core communication.

Layout: the host pre-transposes x per sample (xT [D, T], partition = d)
and casts it to bf16, so the x@W contraction over d maps onto the PE
array with W chunks stationary — no on-chip transpose. Dataflow per
1024-wide t-chunk (2 chunks per sample, 8 per core):
  PE : uitT[e, t] accumulated over 4 K-chunks (bf16, fp32 PSUM)
  ACT: tanh(+ per-partition bias b[e]) PSUM -> SBUF bf16
  PE : ait[1, t] = u-weighted partition reduction (u as weights)
  ACT: ait PSUM -> SBUF row a_row[1, T]
then per sample: a_row -> DRAM bounce -> 0-stride-DMA broadcast to
[128, T]; ACT exp with per-partition accum (softmax denominator lands
in every partition); DVE affine_mul_reduce pools xT * exp directly
into pooled[128, 4]; reciprocal+scale normalizes; DMA out.

The PE stream is software-pipelined one chunk deep: the ait matmuls of
chunk k are interleaved between the uitT groups of chunk k+1 so they
never stall on the tanh (ACT) latency — measured 380 -> 259 ns/matmul.

Bisected-on-HW notes:
 - native DVE TENSOR_TENSOR_REDUCE crashes TRN2
   (NRT_EXEC_UNIT_UNRECOVERABLE); affine_mul_reduce (custom DVE ucode)
   does the same fused multiply+reduce and works.
 - fp32/fp32r moving operands stream at ~2 cycles/column (4-byte
   fetch); bf16 moving operands ~1 cycle/column — hence bf16 matmuls.
 - 0-stride partition-broadcast DMA is legal only from DRAM, so the
   softmax row bounces through a DRAM scratch tile.
"""

import ml_dtypes
import numpy as np

import concourse.bass as bass  # noqa: F401
import concourse.tile as tile
import concourse.mybir as mybir
from concourse import bacc, bass_utils

f32 = mybir.dt.float32
bf16 = mybir.dt.bfloat16
AF = mybir.ActivationFunctionType
ALU = mybir.AluOpType

B, T, D = 32, 2048, 512
NCORES = 8
SPC = B // NCORES        # samples per core
CH = 1024                # t-chunk width (2 PSUM banks)
NCH = SPC * (T // CH)    # pipelined chunks per core (8)
NDC = D // 128           # K-chunks of the contraction (4)
NEC = D // 128           # e-tiles of uitT (4)
EPS = 1e-7


def build():
    nc = bacc.Bacc("TRN2", target_bir_lowering=False, debug=False)

    xT = nc.dram_tensor("xT", [SPC, D, T], bf16, kind="ExternalInput").ap()
    W = nc.dram_tensor("W", [D, D], bf16, kind="ExternalInput").ap()
    b = nc.dram_tensor("b", [D], f32, kind="ExternalInput").ap()
    u = nc.dram_tensor("u", [D], bf16, kind="ExternalInput").ap()
    # out[s, dt, p] == pooled[b=s, d=dt*128+p]; host reshapes to [SPC, D]
    out = nc.dram_tensor("out", [SPC * NDC, 128], f32, kind="ExternalOutput").ap()

    with tile.TileContext(nc) as tc:
        with (
            tc.tile_pool(name="consts", bufs=1) as cpool,
            tc.tile_pool(name="x", bufs=8) as xpool,
            tc.tile_pool(name="th", bufs=8) as thpool,
            tc.tile_pool(name="a", bufs=2) as apool,
            tc.tile_pool(name="s", bufs=2) as spool,
            tc.tile_pool(name="scr", bufs=1) as scrpool,
            tc.tile_pool(name="po", bufs=2) as popool,
            tc.tile_pool(name="dram", bufs=2, space="DRAM") as dpool,
            tc.tile_pool(name="psU", bufs=2, space="PSUM") as psU,
            tc.tile_pool(name="psA", bufs=2, space="PSUM") as psA,
        ):
            xcs = {}         # k -> x chunk tile [128, (dc, CH)]
            th_tiles = {}    # (k, ec) -> bf16 tanh tile [128, CH]
            ait_tiles = {}   # k -> PSUM [1, CH]
            a_bs = {}        # s -> SBUF [128, T] broadcast exp tile
            p8s = {}         # s -> pooled8 [128, 8] partials
            css = {}         # s -> chunksum [128, 2]

            # xT[s] viewed as [p, dc, t] so one DMA loads a whole t-chunk
            xTv = xT.rearrange("s (dc p) t -> s p dc t", p=128)

            def load_chunk(k):
                s, c = k // 2, k % 2
                xc = xpool.tile([128, NDC * CH], bf16, name="xc", tag="xc")
                nc.sync.dma_start(
                    xc[:].rearrange("p (dc t) -> p dc t", t=CH),
                    xTv[s:s + 1, :, :, c * CH:(c + 1) * CH].squeeze(),
                )
                xcs[k] = xc

            # chunk 0's x + weights first: the first matmul group needs them
            load_chunk(0)
            w_sb = cpool.tile([128, NDC * D], bf16)  # [128d, (dc, e)]
            nc.sync.dma_start(
                w_sb[:].rearrange("p (dc e) -> p dc e", e=D),
                W.rearrange("(dc p) e -> p dc e", p=128),
            )
            b_sb = cpool.tile([128, NEC], f32)
            nc.sync.dma_start(b_sb[:], b.rearrange("(c p) -> p c", p=128))
            u_sb = cpool.tile([128, NEC], bf16)
            nc.sync.dma_start(u_sb[:], u.rearrange("(c p) -> p c", p=128))
            for kk in range(1, NCH):
                load_chunk(kk)

            def emit_ait_pair(k, slot):
                """Two of the 8 u-reduction matmuls for chunk k, slot 0-3."""
                for j in range(2):
                    idx = slot * 2 + j
                    h, ec = idx // NEC, idx % NEC
                    hs = slice(h * 512, (h + 1) * 512)
                    nc.tensor.matmul(
                        ait_tiles[k][:, hs], u_sb[:, ec:ec + 1],
                        th_tiles[(k, ec)][:, hs],
                        start=(ec == 0), stop=(ec == NEC - 1),
                    )

            def emit_chunk_tail(k):
                """ait PSUM -> SBUF, broadcast, exp, pooling for chunk k."""
                s, c = k // 2, k % 2
                arow = apool.tile([1, CH], f32, name="a_row", tag="arow")
                nc.scalar.activation(arow[:], ait_tiles[k][:], AF.Copy)
                for ec in range(NEC):
                    del th_tiles[(k, ec)]
                del ait_tiles[k]
                dscr = dpool.tile([1, CH], f32)
                nc.sync.dma_start(dscr[:], arow[:])
                ab = apool.tile([128, CH], f32, tag="ab")
                nc.sync.dma_start(ab[:].unsqueeze(1),
                                  dscr[:].partition_broadcast(128))
                if c == 0:
                    a_bs[s] = apool.tile([128, T], f32, name="a_b", tag="aexp")
                    css[s] = spool.tile([128, 2], f32, name="cs", tag="cs")
                    p8s[s] = popool.tile([128, 2 * NDC], f32, name="p8",
                                         tag="p8")
                csl = slice(c * CH, (c + 1) * CH)
                nc.scalar.activation(a_bs[s][:, csl], ab[:], AF.Exp,
                                     accum_out=css[s][:, c:c + 1])
                for dt in range(NDC):
                    scr = scrpool.tile([128, CH], f32, tag="scr")
                    nc.vector.affine_mul_reduce(
                        out=scr[:], accum_out=p8s[s][:, dt * 2 + c:dt * 2 + c + 1],
                        in0=xcs[k][:, dt * CH:(dt + 1) * CH],
                        in1=a_bs[s][:, csl],
                        scale=1.0, bias=0.0)

            def emit_sample_tail(s):
                S128 = spool.tile([128, 1], f32, tag="S128")
                nc.vector.reduce_sum(S128[:], css[s][:],
                                     axis=mybir.AxisListType.X)
                S128e = spool.tile([128, 1], f32, tag="S128e")
                nc.vector.tensor_scalar_add(S128e[:], S128[:], EPS)
                inv128 = spool.tile([128, 1], f32, tag="inv128")
                nc.vector.reciprocal(inv128[:], S128e[:])
                pooled = popool.tile([128, NDC], f32, tag="pooled")
                nc.vector.reduce_sum(
                    pooled[:],
                    p8s[s][:].rearrange("p (dt h) -> p dt h", dt=NDC),
                    axis=mybir.AxisListType.X)
                pooledn = popool.tile([128, NDC], f32, tag="pooledn")
                nc.vector.tensor_scalar_mul(pooledn[:], pooled[:], inv128[:])
                nc.sync.dma_start(
                    out[s * NDC:(s + 1) * NDC, :].transpose([1, 0]), pooledn[:]
                )

            for k in range(NCH):
                s, c = k // 2, k % 2
                ait_tiles[k] = psA.tile([1, CH], f32, name="ait_ps", tag="ait")
                for ec in range(NEC):
                    ps = psU.tile([128, CH], f32)
                    for h in range(2):
                        for dc in range(NDC):
                            nc.tensor.matmul(
                                ps[:, h * 512:(h + 1) * 512],
                                w_sb[:, dc * D + ec * 128:
                                     dc * D + (ec + 1) * 128],
                                xcs[k][:, dc * CH + h * 512:
                                       dc * CH + h * 512 + 512],
                                start=(dc == 0), stop=(dc == NDC - 1),
                            )
                    th = thpool.tile([128, CH], bf16)
                    nc.scalar.activation(th[:], ps[:], AF.Tanh,
                                         bias=b_sb[:, ec:ec + 1])
                    th_tiles[(k, ec)] = th
                    if k >= 1:
                        emit_ait_pair(k - 1, ec)
                if k == NCH - 1:
                    # final chunk: no next chunk to hide behind; emit now
                    for slot in range(NEC):
                        emit_ait_pair(k, slot)
                if k >= 1:
                    emit_chunk_tail(k - 1)
                    if (k - 1) % 2 == 1:
                        emit_sample_tail((k - 1) // 2)
            emit_chunk_tail(NCH - 1)
            emit_sample_tail(SPC - 1)
    nc.compile()
    return nc


_NC_CACHE = None


def prepare_in_maps(x, W, b, u):
    assert x.shape == (B, T, D) and W.shape == (D, D)
    x = np.ascontiguousarray(x, dtype=np.float32)
    # host-side pre-transpose + bf16 cast: [B, T, D] -> [B, D, T]
    xt = np.ascontiguousarray(
        np.transpose(x, (0, 2, 1)).astype(ml_dtypes.bfloat16))
    W = np.ascontiguousarray(W, dtype=np.float32).astype(ml_dtypes.bfloat16)
    b = np.ascontiguousarray(b, dtype=np.float32)
    u = np.ascontiguousarray(u, dtype=np.float32).astype(ml_dtypes.bfloat16)
    in_maps = []
    for c in range(NCORES):
        shard = xt[c * SPC:(c + 1) * SPC]
        in_maps.append({"xT": shard, "W": W, "b": b, "u": u})
    return in_maps


def kernel(x: np.ndarray, W: np.ndarray, b: np.ndarray, u: np.ndarray) -> np.ndarray:
    global _NC_CACHE
    in_maps = prepare_in_maps(x, W, b, u)

    if _NC_CACHE is None:
        _NC_CACHE = build()
    nc = _NC_CACHE

    res = bass_utils.run_bass_kernel_spmd(
        nc, in_maps, core_ids=list(range(NCORES))
    )
    outs = [r["out"].reshape(SPC, D) for r in res.results]
    return np.concatenate(outs, axis=0).astype(np.float32)


if __name__ == "__main__":
    rng = np.random.default_rng(0)
    x = rng.standard_normal((B, T, D)).astype(np.float32)
    W = (rng.standard_normal((D, D)) / np.sqrt(D)).astype(np.float32)
    b = np.zeros(D, np.float32)
    u = (rng.standard_normal(D) / np.sqrt(D)).astype(np.float32)
    out = kernel(x=x, W=W, b=b, u=u)
    print("out", out.shape, out.dtype, float(np.abs(out).max()))


# revision 12
# speedup vs baseline: 1.0218x; 1.0218x over previous
"""Trainium2 Bass kernel for nn_AttLayer (attention pooling).

Reference computation (per sample b):
    uit = tanh(x @ W + b)            # [T, D]
    ait = uit @ u                    # [T]
    a   = exp(ait); a /= (sum(a) + 1e-7)
    out = a @ x                      # [D]

Sharding: data-parallel over batch B=32 across 8 cores (4 samples/core);
W/b/u replicated. No cross-core communication.

Layout: the host pre-transposes x per sample (xT [D, T], partition = d)
and casts it to bf16, so the x@W contraction over d maps onto the PE
array with W chunks stationary — no on-chip transpose. Dataflow per
1024-wide t-chunk (2 chunks per sample, 8 per core):
  PE : uitT[e, t] accumulated over 4 K-chunks (bf16, fp32 PSUM)
  ACT: tanh(+ per-partition bias b[e]) PSUM -> SBUF bf16
  PE : ait[1, t] = u-weighted partition reduction (u as weights)
  ACT: ait PSUM -> SBUF row a_row[1, T]
then per sample: a_row -> DRAM bounce -> 0-stride-DMA broadcast to
[128, T]; ACT exp with per-partition accum (softmax denominator lands
in every partition); DVE affine_mul_reduce pools xT * exp directly
into pooled[128, 4]; reciprocal+scale normalizes; DMA out.

The PE stream is software-pipelined one chunk deep: the ait matmuls of
chunk k are interleaved between the uitT groups of chunk k+1 so they
never stall on the tanh (ACT) latency — measured 380 -> 259 ns/matmul.

Bisected-on-HW notes:
 - native DVE TENSOR_TENSOR_REDUCE crashes TRN2
   (NRT_EXEC_UNIT_UNRECOVERABLE); affine_mul_reduce (custom DVE ucode)
   does the same fused multiply+reduce and works.
 - fp32/fp32r moving operands stream at ~2 cycles/column (4-byte
   fetch); bf16 moving operands ~1 cycle/column — hence bf16 matmuls.
 - 0-stride partition-broadcast DMA is legal only from DRAM, so the
   softmax row bounces through a DRAM scratch tile.
"""

import ml_dtypes
import numpy as np

import concourse.bass as bass  # noqa: F401
import concourse.tile as tile
import concourse.mybir as mybir
from concourse import bacc, bass_utils

f32 = mybir.dt.float32
bf16 = mybir.dt.bfloat16
AF = mybir.ActivationFunctionType
ALU = mybir.AluOpType

B, T, D = 32, 2048, 512
NCORES = 8
SPC = B // NCORES        # samples per core
CH = 1024                # t-chunk width (2 PSUM banks)
NCH = SPC * (T // CH)    # pipelined chunks per core (8)
NDC = D // 128           # K-chunks of the contraction (4)
NEC = D // 128           # e-tiles of uitT (4)
EPS = 1e-7


def build():
    nc = bacc.Bacc("TRN2", target_bir_lowering=False, debug=False)

    xT = nc.dram_tensor("xT", [SPC, D, T], bf16, kind="ExternalInput").ap()
    W = nc.dram_tensor("W", [D, D], bf16, kind="ExternalInput").ap()
    b = nc.dram_tensor("b", [D], f32, kind="ExternalInput").ap()
    u = nc.dram_tensor("u", [D], bf16, kind="ExternalInput").ap()
    ones = nc.dram_tensor("ones", [1, 128], mybir.dt.float32r,
                          kind="ExternalInput").ap()
    # out[s, dt, p] == pooled[b=s, d=dt*128+p]; host reshapes to [SPC, D]
    out = nc.dram_tensor("out", [SPC * NDC, 128], f32, kind="ExternalOutput").ap()

    with tile.TileContext(nc) as tc:
        with (
            tc.tile_pool(name="consts", bufs=1) as cpool,
            tc.tile_pool(name="x", bufs=8) as xpool,
            tc.tile_pool(name="th", bufs=8) as thpool,
            tc.tile_pool(name="a", bufs=2) as apool,
            tc.tile_pool(name="s", bufs=2) as spool,
            tc.tile_pool(name="scr", bufs=1) as scrpool,
            tc.tile_pool(name="po", bufs=2) as popool,
            tc.tile_pool(name="psU", bufs=2, space="PSUM") as psU,
            tc.tile_pool(name="psA", bufs=2, space="PSUM") as psA,
        ):
            xcs = {}         # k -> x chunk tile [128, (dc, CH)]
            th_tiles = {}    # (k, ec) -> bf16 tanh tile [128, CH]
            ait_tiles = {}   # k -> PSUM [1, CH]
            a_bs = {}        # s -> SBUF [128, T] broadcast exp tile
            arows = {}       # k -> SBUF [1, CH] ait row (f32r)
            p8s = {}         # s -> pooled8 [128, 8] partials
            css = {}         # s -> chunksum [128, 2]

            # xT[s] viewed as [p, dc, t] so one DMA loads a whole t-chunk
            xTv = xT.rearrange("s (dc p) t -> s p dc t", p=128)

            def load_chunk(k):
                s, c = k // 2, k % 2
                xc = xpool.tile([128, NDC * CH], bf16, name="xc", tag="xc")
                nc.sync.dma_start(
                    xc[:].rearrange("p (dc t) -> p dc t", t=CH),
                    xTv[s:s + 1, :, :, c * CH:(c + 1) * CH].squeeze(),
                )
                xcs[k] = xc

            # chunk 0's x + weights first: the first matmul group needs them
            load_chunk(0)
            w_sb = cpool.tile([128, NDC * D], bf16)  # [128d, (dc, e)]
            nc.sync.dma_start(
                w_sb[:].rearrange("p (dc e) -> p dc e", e=D),
                W.rearrange("(dc p) e -> p dc e", p=128),
            )
            b_sb = cpool.tile([128, NEC], f32)
            nc.sync.dma_start(b_sb[:], b.rearrange("(c p) -> p c", p=128))
            u_sb = cpool.tile([128, NEC], bf16)
            nc.sync.dma_start(u_sb[:], u.rearrange("(c p) -> p c", p=128))
            ones_sb = cpool.tile([1, 128], mybir.dt.float32r)
            nc.sync.dma_start(ones_sb[:], ones[:])
            for kk in range(1, NCH):
                load_chunk(kk)

            def emit_ait_pair(k, slot):
                """Two of the 8 u-reduction matmuls for chunk k, slot 0-3."""
                for j in range(2):
                    idx = slot * 2 + j
                    h, ec = idx // NEC, idx % NEC
                    hs = slice(h * 512, (h + 1) * 512)
                    nc.tensor.matmul(
                        ait_tiles[k][0:1, hs], u_sb[:, ec:ec + 1],
                        th_tiles[(k, ec)][:, hs],
                        start=(ec == 0), stop=(ec == NEC - 1),
                    )

            def emit_aitc(k):
                s, c = k // 2, k % 2
                arow = apool.tile([1, CH], mybir.dt.float32r, name="a_row",
                                  tag="arow")
                nc.scalar.activation(arow[:], ait_tiles[k][0:1, :], AF.Copy)
                arows[k] = arow
                for ec in range(NEC):
                    del th_tiles[(k, ec)]

            def emit_bcast(k):
                # broadcast ait row across partitions, in place in PSUM
                for n in range(2):
                    nc.tensor.matmul(
                        ait_tiles[k][:, n * 512:(n + 1) * 512], ones_sb[:],
                        arows[k][:, n * 512:(n + 1) * 512],
                        start=True, stop=True)

            def emit_exp_pool(k):
                s, c = k // 2, k % 2
                if c == 0:
                    a_bs[s] = apool.tile([128, T], f32, name="a_b", tag="aexp")
                    css[s] = spool.tile([128, 2], f32, name="cs", tag="cs")
                    p8s[s] = popool.tile([128, 2 * NDC], f32, name="p8",
                                         tag="p8")
                csl = slice(c * CH, (c + 1) * CH)
                nc.scalar.activation(a_bs[s][:, csl], ait_tiles[k][:], AF.Exp,
                                     accum_out=css[s][:, c:c + 1])
                del ait_tiles[k]
                del arows[k]
                for dt in range(NDC):
                    scr = scrpool.tile([128, CH], f32, tag="scr")
                    nc.vector.affine_mul_reduce(
                        out=scr[:], accum_out=p8s[s][:, dt * 2 + c:dt * 2 + c + 1],
                        in0=xcs[k][:, dt * CH:(dt + 1) * CH],
                        in1=a_bs[s][:, csl],
                        scale=1.0, bias=0.0)

            def emit_sample_tail(s):
                S128 = spool.tile([128, 1], f32, tag="S128")
                nc.vector.reduce_sum(S128[:], css[s][:],
                                     axis=mybir.AxisListType.X)
                S128e = spool.tile([128, 1], f32, tag="S128e")
                nc.vector.tensor_scalar_add(S128e[:], S128[:], EPS)
                inv128 = spool.tile([128, 1], f32, tag="inv128")
                nc.vector.reciprocal(inv128[:], S128e[:])
                pooled = popool.tile([128, NDC], f32, tag="pooled")
                nc.vector.reduce_sum(
                    pooled[:],
                    p8s[s][:].rearrange("p (dt h) -> p dt h", dt=NDC),
                    axis=mybir.AxisListType.X)
                pooledn = popool.tile([128, NDC], f32, tag="pooledn")
                nc.vector.tensor_scalar_mul(pooledn[:], pooled[:], inv128[:])
                nc.sync.dma_start(
                    out[s * NDC:(s + 1) * NDC, :].transpose([1, 0]), pooledn[:]
                )

            for k in range(NCH):
                s, c = k // 2, k % 2
                ait_tiles[k] = psA.tile([128, CH], f32, name="ait_ps",
                                        tag="ait")
                for ec in range(NEC):
                    ps = psU.tile([128, CH], f32)
                    for h in range(2):
                        for dc in range(NDC):
                            nc.tensor.matmul(
                                ps[:, h * 512:(h + 1) * 512],
                                w_sb[:, dc * D + ec * 128:
                                     dc * D + (ec + 1) * 128],
                                xcs[k][:, dc * CH + h * 512:
                                       dc * CH + h * 512 + 512],
                                start=(dc == 0), stop=(dc == NDC - 1),
                            )
                    th = thpool.tile([128, CH], bf16)
                    nc.scalar.activation(th[:], ps[:], AF.Tanh,
                                         bias=b_sb[:, ec:ec + 1])
                    th_tiles[(k, ec)] = th
                    if k >= 1:
                        # staged pipeline for chunk k-1's tail
                        if ec == 0:
                            emit_ait_pair(k - 1, 0)
                            emit_ait_pair(k - 1, 1)
                        elif ec == 1:
                            emit_ait_pair(k - 1, 2)
                            emit_ait_pair(k - 1, 3)
                            emit_aitc(k - 1)
                        elif ec == 2:
                            emit_bcast(k - 1)
                        else:
                            emit_exp_pool(k - 1)
                            if (k - 1) % 2 == 1:
                                emit_sample_tail((k - 1) // 2)
            # epilogue for the final chunk
            kf = NCH - 1
            for slot in range(NEC):
                emit_ait_pair(kf, slot)
            emit_aitc(kf)
            emit_bcast(kf)
            emit_exp_pool(kf)
            emit_sample_tail(SPC - 1)
    nc.compile()
    return nc


_NC_CACHE = None


def prepare_in_maps(x, W, b, u):
    assert x.shape == (B, T, D) and W.shape == (D, D)
    x = np.ascontiguousarray(x, dtype=np.float32)
    # host-side pre-transpose + bf16 cast: [B, T, D] -> [B, D, T]
    xt = np.ascontiguousarray(
        np.transpose(x, (0, 2, 1)).astype(ml_dtypes.bfloat16))
    W = np.ascontiguousarray(W, dtype=np.float32).astype(ml_dtypes.bfloat16)
    b = np.ascontiguousarray(b, dtype=np.float32)
    u = np.ascontiguousarray(u, dtype=np.float32).astype(ml_dtypes.bfloat16)
    ones = np.ones((1, 128), dtype=np.float32)
    in_maps = []
    for c in range(NCORES):
        shard = xt[c * SPC:(c + 1) * SPC]
        in_maps.append({"xT": shard, "W": W, "b": b, "u": u, "ones": ones})
    return in_maps


def kernel(x: np.ndarray, W: np.ndarray, b: np.ndarray, u: np.ndarray) -> np.ndarray:
    global _NC_CACHE
    in_maps = prepare_in_maps(x, W, b, u)

    if _NC_CACHE is None:
        _NC_CACHE = build()
    nc = _NC_CACHE

    res = bass_utils.run_bass_kernel_spmd(
        nc, in_maps, core_ids=list(range(NCORES))
    )
    outs = [r["out"].reshape(SPC, D) for r in res.results]
    return np.concatenate(outs, axis=0).astype(np.float32)


if __name__ == "__main__":
    rng = np.random.default_rng(0)
    x = rng.standard_normal((B, T, D)).astype(np.float32)
    W = (rng.standard_normal((D, D)) / np.sqrt(D)).astype(np.float32)
    b = np.zeros(D, np.float32)
    u = (rng.standard_normal(D) / np.sqrt(D)).astype(np.float32)
    out = kernel(x=x, W=W, b=b, u=u)
    print("out", out.shape, out.dtype, float(np.abs(out).max()))


# revision 13
# speedup vs baseline: 1.0311x; 1.0091x over previous
"""Trainium2 Bass kernel for nn_AttLayer (attention pooling).

Reference computation (per sample b):
    uit = tanh(x @ W + b)            # [T, D]
    ait = uit @ u                    # [T]
    a   = exp(ait); a /= (sum(a) + 1e-7)
    out = a @ x                      # [D]

Sharding: data-parallel over batch B=32 across 8 cores (4 samples/core);
W/b/u replicated. No cross-core communication.

Layout: the host pre-transposes x per sample (xT [D, T], partition = d)
and casts it to bf16, so the x@W contraction over d maps onto the PE
array with W chunks stationary — no on-chip transpose. Dataflow per
1024-wide t-chunk (2 chunks per sample, 8 per core):
  PE : uitT[e, t] accumulated over 4 K-chunks (bf16, fp32 PSUM)
  ACT: tanh(+ per-partition bias b[e]) PSUM -> SBUF bf16
  PE : ait[1, t] = u-weighted partition reduction (u as weights)
  ACT: ait PSUM -> SBUF row a_row[1, T]
then per sample: a_row -> DRAM bounce -> 0-stride-DMA broadcast to
[128, T]; ACT exp with per-partition accum (softmax denominator lands
in every partition); DVE affine_mul_reduce pools xT * exp directly
into pooled[128, 4]; reciprocal+scale normalizes; DMA out.

The PE stream is software-pipelined one chunk deep: the ait matmuls of
chunk k are interleaved between the uitT groups of chunk k+1 so they
never stall on the tanh (ACT) latency — measured 380 -> 259 ns/matmul.

Bisected-on-HW notes:
 - native DVE TENSOR_TENSOR_REDUCE crashes TRN2
   (NRT_EXEC_UNIT_UNRECOVERABLE); affine_mul_reduce (custom DVE ucode)
   does the same fused multiply+reduce and works.
 - fp32/fp32r moving operands stream at ~2 cycles/column (4-byte
   fetch); bf16 moving operands ~1 cycle/column — hence bf16 matmuls.
 - 0-stride partition-broadcast DMA is legal only from DRAM, so the
   softmax row bounces through a DRAM scratch tile.
"""

import ml_dtypes
import numpy as np

import concourse.bass as bass  # noqa: F401
import concourse.tile as tile
import concourse.mybir as mybir
from concourse import bacc, bass_utils

f32 = mybir.dt.float32
bf16 = mybir.dt.bfloat16
AF = mybir.ActivationFunctionType
ALU = mybir.AluOpType

B, T, D = 32, 2048, 512
NCORES = 8
SPC = B // NCORES        # samples per core
CH = 1024                # t-chunk width (2 PSUM banks)
NCH = SPC * (T // CH)    # pipelined chunks per core (8)
NDC = D // 128           # K-chunks of the contraction (4)
NEC = D // 128           # e-tiles of uitT (4)
EPS = 1e-7


def build():
    nc = bacc.Bacc("TRN2", target_bir_lowering=False, debug=False)

    xT = nc.dram_tensor("xT", [SPC, D, T], bf16, kind="ExternalInput").ap()
    W = nc.dram_tensor("W", [D, D], bf16, kind="ExternalInput").ap()
    b = nc.dram_tensor("b", [D], f32, kind="ExternalInput").ap()
    u = nc.dram_tensor("u", [D], bf16, kind="ExternalInput").ap()
    # out[s, dt, p] == pooled[b=s, d=dt*128+p]; host reshapes to [SPC, D]
    out = nc.dram_tensor("out", [SPC * NDC, 128], f32, kind="ExternalOutput").ap()

    with tile.TileContext(nc) as tc:
        with (
            tc.tile_pool(name="consts", bufs=1) as cpool,
            tc.tile_pool(name="x", bufs=4) as xpool,
            tc.tile_pool(name="th", bufs=8) as thpool,
            tc.tile_pool(name="a", bufs=2) as apool,
            tc.tile_pool(name="s", bufs=2) as spool,
            tc.tile_pool(name="scr", bufs=1) as scrpool,
            tc.tile_pool(name="po", bufs=2) as popool,
            tc.tile_pool(name="dram", bufs=2, space="DRAM") as dpool,
            tc.tile_pool(name="psU", bufs=2, space="PSUM") as psU,
            tc.tile_pool(name="psA", bufs=2, space="PSUM") as psA,
        ):
            xts_all = {}     # s -> [4 xT tiles]
            th_tiles = {}    # (k, ec) -> bf16 tanh tile [128, CH]
            ait_tiles = {}   # k -> PSUM [1, CH]
            a_bs = {}        # s -> SBUF [128, T] broadcast exp tile
            p8s = {}         # s -> pooled8 [128, 8] partials
            css = {}         # s -> chunksum [128, 2]

            def load_sample(s):
                tiles = []
                for dc in range(NDC):
                    xt = xpool.tile([128, T], bf16, tag=f"x{dc}")
                    nc.sync.dma_start(xt[:], xT[s, dc * 128:(dc + 1) * 128, :])
                    tiles.append(xt)
                xts_all[s] = tiles

            # sample 0's x first: the first matmul group needs it
            load_sample(0)

            # ---- constants (loaded once) ----
            w_sb = cpool.tile([128, NDC * D], bf16)  # [128d, (dc, e)]
            for dc in range(NDC):
                nc.sync.dma_start(w_sb[:, dc * D:(dc + 1) * D],
                                  W[dc * 128:(dc + 1) * 128, :])
            b_sb = cpool.tile([128, NEC], f32)
            nc.sync.dma_start(b_sb[:], b.rearrange("(c p) -> p c", p=128))
            u_sb = cpool.tile([128, NEC], bf16)
            nc.sync.dma_start(u_sb[:], u.rearrange("(c p) -> p c", p=128))
            for _s in range(1, SPC):
                load_sample(_s)

            def emit_ait_pair(k, slot):
                """Two of the 8 u-reduction matmuls for chunk k, slot 0-3."""
                for j in range(2):
                    idx = slot * 2 + j
                    h, ec = idx // NEC, idx % NEC
                    hs = slice(h * 512, (h + 1) * 512)
                    nc.tensor.matmul(
                        ait_tiles[k][:, hs], u_sb[:, ec:ec + 1],
                        th_tiles[(k, ec)][:, hs],
                        start=(ec == 0), stop=(ec == NEC - 1),
                    )

            def emit_chunk_tail(k):
                """ait PSUM -> SBUF, broadcast, exp, pooling for chunk k."""
                s, c = k // 2, k % 2
                arow = apool.tile([1, CH], f32, name="a_row", tag="arow")
                nc.scalar.activation(arow[:], ait_tiles[k][:], AF.Copy)
                for ec in range(NEC):
                    del th_tiles[(k, ec)]
                del ait_tiles[k]
                dscr = dpool.tile([1, CH], f32)
                nc.sync.dma_start(dscr[:], arow[:])
                ab = apool.tile([128, CH], f32, tag="ab")
                nc.sync.dma_start(ab[:].unsqueeze(1),
                                  dscr[:].partition_broadcast(128))
                if c == 0:
                    a_bs[s] = apool.tile([128, T], f32, name="a_b", tag="aexp")
                    css[s] = spool.tile([128, 2], f32, name="cs", tag="cs")
                    p8s[s] = popool.tile([128, 2 * NDC], f32, name="p8",
                                         tag="p8")
                csl = slice(c * CH, (c + 1) * CH)
                nc.scalar.activation(a_bs[s][:, csl], ab[:], AF.Exp,
                                     accum_out=css[s][:, c:c + 1])
                for dt in range(NDC):
                    scr = scrpool.tile([128, CH], f32, tag="scr")
                    nc.vector.affine_mul_reduce(
                        out=scr[:], accum_out=p8s[s][:, dt * 2 + c:dt * 2 + c + 1],
                        in0=xts_all[s][dt][:, csl], in1=a_bs[s][:, csl],
                        scale=1.0, bias=0.0)

            def emit_sample_tail(s):
                S128 = spool.tile([128, 1], f32, tag="S128")
                nc.vector.reduce_sum(S128[:], css[s][:],
                                     axis=mybir.AxisListType.X)
                S128e = spool.tile([128, 1], f32, tag="S128e")
                nc.vector.tensor_scalar_add(S128e[:], S128[:], EPS)
                inv128 = spool.tile([128, 1], f32, tag="inv128")
                nc.vector.reciprocal(inv128[:], S128e[:])
                pooled = popool.tile([128, NDC], f32, tag="pooled")
                nc.vector.reduce_sum(
                    pooled[:],
                    p8s[s][:].rearrange("p (dt h) -> p dt h", dt=NDC),
                    axis=mybir.AxisListType.X)
                pooledn = popool.tile([128, NDC], f32, tag="pooledn")
                nc.vector.tensor_scalar_mul(pooledn[:], pooled[:], inv128[:])
                nc.sync.dma_start(
                    out[s * NDC:(s + 1) * NDC, :].transpose([1, 0]), pooledn[:]
                )

            for k in range(NCH):
                s, c = k // 2, k % 2
                ait_tiles[k] = psA.tile([1, CH], f32, name="ait_ps", tag="ait")
                for ec in range(NEC):
                    ps = psU.tile([128, CH], f32)
                    for h in range(2):
                        toff = c * CH + h * 512
                        for dc in range(NDC):
                            nc.tensor.matmul(
                                ps[:, h * 512:(h + 1) * 512],
                                w_sb[:, dc * D + ec * 128:
                                     dc * D + (ec + 1) * 128],
                                xts_all[s][dc][:, toff:toff + 512],
                                start=(dc == 0), stop=(dc == NDC - 1),
                            )
                    th = thpool.tile([128, CH], bf16)
                    nc.scalar.activation(th[:], ps[:], AF.Tanh,
                                         bias=b_sb[:, ec:ec + 1])
                    th_tiles[(k, ec)] = th
                    if k >= 1:
                        emit_ait_pair(k - 1, ec)
                if k == NCH - 1:
                    # final chunk: no next chunk to hide behind; emit now
                    for slot in range(NEC):
                        emit_ait_pair(k, slot)
                if k >= 1:
                    emit_chunk_tail(k - 1)
                    if (k - 1) % 2 == 1:
                        emit_sample_tail((k - 1) // 2)
            emit_chunk_tail(NCH - 1)
            emit_sample_tail(SPC - 1)
    nc.compile()
    return nc


_NC_CACHE = None


def prepare_in_maps(x, W, b, u):
    assert x.shape == (B, T, D) and W.shape == (D, D)
    x = np.ascontiguousarray(x, dtype=np.float32)
    # host-side pre-transpose + bf16 cast: [B, T, D] -> [B, D, T]
    xt = np.ascontiguousarray(
        np.transpose(x, (0, 2, 1)).astype(ml_dtypes.bfloat16))
    W = np.ascontiguousarray(W, dtype=np.float32).astype(ml_dtypes.bfloat16)
    b = np.ascontiguousarray(b, dtype=np.float32)
    u = np.ascontiguousarray(u, dtype=np.float32).astype(ml_dtypes.bfloat16)
    in_maps = []
    for c in range(NCORES):
        shard = xt[c * SPC:(c + 1) * SPC]
        in_maps.append({"xT": shard, "W": W, "b": b, "u": u})
    return in_maps


def kernel(x: np.ndarray, W: np.ndarray, b: np.ndarray, u: np.ndarray) -> np.ndarray:
    global _NC_CACHE
    in_maps = prepare_in_maps(x, W, b, u)

    if _NC_CACHE is None:
        _NC_CACHE = build()
    nc = _NC_CACHE

    res = bass_utils.run_bass_kernel_spmd(
        nc, in_maps, core_ids=list(range(NCORES))
    )
    outs = [r["out"].reshape(SPC, D) for r in res.results]
    return np.concatenate(outs, axis=0).astype(np.float32)


if __name__ == "__main__":
    rng = np.random.default_rng(0)
    x = rng.standard_normal((B, T, D)).astype(np.float32)
    W = (rng.standard_normal((D, D)) / np.sqrt(D)).astype(np.float32)
    b = np.zeros(D, np.float32)
    u = (rng.standard_normal(D) / np.sqrt(D)).astype(np.float32)
    out = kernel(x=x, W=W, b=b, u=u)
    print("out", out.shape, out.dtype, float(np.abs(out).max()))
